# revision 1
# baseline (speedup 1.0000x reference)
"""GAT message-passing kernel for Trainium2 (8 NeuronCores, Bass/Tile).

Strategy (edge/graph parallelism, per the sharding hint):
  - Host: sort edges by dst, split dst-node space into 8 contiguous ranges with
    ~equal edge counts (one range per core). Per core, pack nodes into "blocks"
    of <=S nodes and <=KC*128 edge slots; edges of a block are padded to KC
    128-edge chunks. All numerical work happens on device; the host only
    rearranges indices (sharding) and reassembles rows (unsharding).
  - Device phase 1 (replicated on all cores): T = [feat @ fc_w | el | er]
    written to a DRAM table (N x 136).
  - Device phase 2 (sharded by edges/dst): per 128-edge chunk, indirect-gather
    T rows by src (feat_src + el) and er by dst; compute
    ex = exp(leaky_relu(el[src] + er[dst])); build a one-hot edge->slot matrix P
    from precomputed slot ids; matmul P^T @ [feat_src * ex | ex] accumulated in
    PSUM per block. Epilogue divides by the per-(node,head) denominator
    (the segment softmax normalization) and streams rows out contiguously.
"""

import math
import os
import numpy as np

# ---------------- problem constants (hardcoded; kernel.py is self-contained) ---
N = 100000
F = 128           # input feature dim (= contraction dim)
H = 4             # heads
D = 32            # dim per head
HD = H * D        # 128
TCOLS = F + 2 * H  # 136 = feat_src(128) + el(4) + er(4)
ML = HD + H       # 132 = msg cols + ex cols
NEG = 0.2
NCORES = 8

# ---------------- device tiling parameters ------------------------------------
S = 32            # node slots per block
KC = 4            # 128-edge chunks per block
CHE = 128         # edges per chunk
BSLOTS = KC * CHE # 512 edge slots per block
SUP_B = 4         # blocks per supertile
SUP_CH = SUP_B * KC
PAD_SEG = 100000  # slot id for padding edges (matches no one-hot column)

# phase-1 layout
NPAD = 100352         # 784 tiles of 128 node rows (>= N)
WCH = 2048            # featT columns loaded per DMA (16 tiles)
G1 = 8                # node tiles per T write


def _pack(src, dst, n_cores, n_nodes, n_edges):
    """Host-side index preprocessing. Returns (edata list, node_map list, B)."""
    order = np.argsort(dst, kind="stable")
    s_src = np.asarray(src, np.int64)[order]
    s_dst = np.asarray(dst, np.int64)[order]
    deg = np.bincount(dst, minlength=n_nodes).astype(np.int64)
    assert deg.max() <= BSLOTS, "node degree exceeds block capacity"
    cum = np.cumsum(deg)
    estart = cum - deg
    bnd = [0]
    for k in range(1, n_cores):
        bnd.append(int(np.searchsorted(cum, n_edges * k / n_cores)))
    bnd.append(n_nodes)

    node_block = np.zeros(n_nodes, np.int64)
    node_slot = np.zeros(n_nodes, np.int64)
    nblocks = []
    for k in range(n_cores):
        nb = 0
        cnt = 0
        slots = 0
        for n in range(bnd[k], bnd[k + 1]):
            d = deg[n]
            if cnt >= S or slots + d > BSLOTS:
                nb += 1
                cnt = 0
                slots = 0
            node_block[n] = nb
            node_slot[n] = cnt
            cnt += 1
            slots += d
        nblocks.append(nb + 1 if bnd[k + 1] > bnd[k] else 0)
    B = max(nblocks)
    B = int(math.ceil(B / SUP_B) * SUP_B)

    edatas = []
    node_maps = []
    for k in range(n_cores):
        lo, hi = bnd[k], bnd[k + 1]
        e_lo = int(estart[lo]) if lo < n_nodes else n_edges
        e_hi = int(estart[hi]) if hi < n_nodes else n_edges
        ksrc = s_src[e_lo:e_hi]
        kdst = s_dst[e_lo:e_hi]
        kblk = node_block[kdst]
        kslot = node_slot[kdst]
        # first sorted-edge index of each block (via its first node)
        nodes = np.arange(lo, hi)
        blk_of_node = node_block[lo:hi]
        nb_k = nblocks[k]
        first_edge = np.zeros(max(nb_k, 1), np.int64)
        ub, ui = np.unique(blk_of_node, return_index=True)
        first_edge[ub] = estart[nodes[ui]]
        pos = np.arange(e_lo, e_hi) - first_edge[kblk]
        assert pos.max(initial=0) < BSLOTS
        c = kblk * KC + pos // CHE
        p = pos % CHE
        # per-supertile layout: [src cols | seg cols | dst cols], each
        # SUP_CH wide and contiguous (indirect-DMA offset APs must be
        # contiguous in the last dim)
        sp_of_c = c // SUP_CH
        i_of_c = c % SUP_CH
        base = sp_of_c * 3 * SUP_CH
        ed = np.zeros((CHE, B * KC * 3), np.int32)
        seg_cols = (np.arange(B * KC * 3)
                    .reshape(-1, 3 * SUP_CH)[:, SUP_CH:2 * SUP_CH].reshape(-1))
        ed[:, seg_cols] = PAD_SEG
        ed[p, base + i_of_c] = ksrc
        ed[p, base + SUP_CH + i_of_c] = kslot
        ed[p, base + 2 * SUP_CH + i_of_c] = kdst
        nm = np.full(B * S, -1, np.int64)
        nm[blk_of_node * S + node_slot[lo:hi]] = nodes
        # per-supertile slot -> node id (for the per-supertile er gather);
        # pad slots point at row 0 (gathered junk is never read back)
        nsup_k = B // SUP_B
        nid = np.zeros((SUP_B * S, nsup_k), np.int32)
        nid[(blk_of_node % SUP_B) * S + node_slot[lo:hi],
            blk_of_node // SUP_B] = nodes
        # block-slot of each edge replicated across S partitions, fp32,
        # laid out [S, nchunks*CHE] so PT one-hots build via tensor_scalar
        segT = np.full((1, B * KC * CHE), float(PAD_SEG), np.float32)
        segT[0, c * CHE + p] = (kslot + S * (kblk % SUP_B)).astype(np.float32)
        segT = np.broadcast_to(segT, (SUP_B * S, B * KC * CHE)).copy()
        edatas.append((ed, nid, segT))
        node_maps.append(nm)
    return edatas, node_maps, B


def _build(B, npad=NPAD, wch=WCH, g1=G1):
    """Build the per-core Bass program (identical across cores)."""
    import concourse.bacc as bacc
    import concourse.tile as tile
    import concourse.mybir as mybir
    from concourse.bass import IndirectOffsetOnAxis

    F32 = mybir.dt.float32
    I32 = mybir.dt.int32
    AOT = mybir.AluOpType

    nsup = B // SUP_B
    n_tiles = npad // 128

    nc = bacc.Bacc("TRN2", target_bir_lowering=False, debug=False)
    featT = nc.dram_tensor("featT", [F, npad], F32, kind="ExternalInput")
    fcw = nc.dram_tensor("fcw", [F, HD], F32, kind="ExternalInput")
    attn = nc.dram_tensor("attn", [1, 2 * HD], F32, kind="ExternalInput")
    edata = nc.dram_tensor("edata", [CHE, B * KC * 3], I32, kind="ExternalInput")
    nid_d = nc.dram_tensor("nid", [SUP_B * S, B // SUP_B], I32, kind="ExternalInput")
    segT_d = nc.dram_tensor("segT", [SUP_B * S, B * KC * CHE], F32, kind="ExternalInput")
    T = nc.dram_tensor("T", [npad, TCOLS], F32, kind="Internal")
    out = nc.dram_tensor("out", [B * S, HD], F32, kind="ExternalOutput")

    with tile.TileContext(nc) as tc:
        with tc.tile_pool(name="const", bufs=1) as const:
            # ---- weight prep: W_aug = [fc_w | W_l | W_r] ----
            w_aug = const.tile([F, TCOLS], F32)
            nc.sync.dma_start(out=w_aug[:, 0:HD], in_=fcw[:, :])
            attn_sb = const.tile([1, 2 * HD], F32)
            nc.sync.dma_start(out=attn_sb[:], in_=attn[:, :])
            ab = const.tile([F, 2 * HD], F32)
            nc.gpsimd.partition_broadcast(ab[:], attn_sb[:])
            tmp = const.tile([F, 2 * HD], F32)
            nc.vector.tensor_tensor(
                out=tmp[:].rearrange("p (t w) -> p t w", t=2),
                in0=w_aug[:, None, 0:HD].broadcast_to([F, 2, HD]),
                in1=ab[:].rearrange("p (t w) -> p t w", t=2),
                op=AOT.mult,
            )
            nc.vector.tensor_reduce(
                w_aug[:, HD:HD + 2 * H].rearrange("p (t h) -> p t h", t=2),
                tmp[:].rearrange("p (t h d) -> p t h d", t=2, h=H),
                mybir.AxisListType.X,
                AOT.add,
            )

            # ---- phase 1: T = [feat @ W_aug] ----
            with tc.tile_pool(name="fp", bufs=3) as fpool, \
                 tc.tile_pool(name="p1ps", bufs=8, space="PSUM") as p1ps, \
                 tc.tile_pool(name="st1", bufs=4) as st1p:
                tpw = wch // 128
                for w in range(npad // wch):
                    fsb = fpool.tile([F, wch], F32)
                    nc.sync.dma_start(out=fsb[:], in_=featT[:, w * wch:(w + 1) * wch])
                    for grp in range(tpw // g1):
                        stg = st1p.tile([F, g1 * TCOLS], F32)
                        for j in range(g1):
                            ps = p1ps.tile([128, TCOLS], F32)
                            col0 = (grp * g1 + j) * 128
                            nc.tensor.matmul(
                                out=ps[:],
                                lhsT=fsb[:, col0:col0 + 128],
                                rhs=w_aug[:],
                                start=True, stop=True,
                            )
                            nc.vector.tensor_copy(
                                out=stg[:, j * TCOLS:(j + 1) * TCOLS], in_=ps[:]
                            )
                        t0 = w * tpw + grp * g1
                        nc.sync.dma_start(
                            out=T[t0 * 128:(t0 + g1) * 128, :].rearrange(
                                "(j p) c -> p j c", j=g1),
                            in_=stg[:].rearrange("p (j c) -> p j c", j=g1),
                        )

            # ---- phase 2: edge processing ----
            iot = const.tile([CHE, S], I32)
            nc.gpsimd.iota(iot[:], pattern=[[1, S]], base=0, channel_multiplier=0)
            iot_col = const.tile([SUP_B * S, 1], I32)
            nc.gpsimd.iota(iot_col[:], pattern=[[0, 1]], base=0,
                           channel_multiplier=1)
            iot_colf = const.tile([SUP_B * S, 1], F32)
            nc.vector.tensor_copy(out=iot_colf[:], in_=iot_col[:])
            nid_sb = const.tile([SUP_B * S, nsup], I32)
            nc.sync.dma_start(out=nid_sb[:], in_=nid_d[:, :])

            with tc.tile_pool(name="ed", bufs=5) as edp, \
                 tc.tile_pool(name="gg", bufs=5) as gp, \
                 tc.tile_pool(name="sgt", bufs=4) as sgtp, \
                 tc.tile_pool(name="ers", bufs=6) as ersp, \
                 tc.tile_pool(name="pp", bufs=3) as ppool, \
                 tc.tile_pool(name="pt", bufs=4) as ptp, \
                 tc.tile_pool(name="mx", bufs=3) as mxp, \
                 tc.tile_pool(name="exu", bufs=3) as exup, \
                 tc.tile_pool(name="rr", bufs=8) as rp, \
                 tc.tile_pool(name="st2", bufs=3) as st2p, \
                 tc.tile_pool(name="p2ps", bufs=6, space="PSUM") as p2ps, \
                 tc.tile_pool(name="erps", bufs=2, space="PSUM") as erps:
                for sp in range(nsup):
                    ed = edp.tile([CHE, SUP_CH * 3], I32)
                    nc.sync.dma_start(
                        out=ed[:],
                        in_=edata[:, sp * SUP_CH * 3:(sp + 1) * SUP_CH * 3])
                    ed_src = ed[:, 0:SUP_CH]
                    ed_seg = ed[:, SUP_CH:2 * SUP_CH]

                    sgt = sgtp.tile([SUP_B * S, SUP_CH * CHE], F32)
                    nc.sync.dma_start(
                        out=sgt[:],
                        in_=segT_d[:, sp * SUP_CH * CHE:(sp + 1) * SUP_CH * CHE])

                    # per-supertile er gather: one index per slot (node id)
                    er_sup = ersp.tile([SUP_B * S, H], F32)
                    nc.gpsimd.indirect_dma_start(
                        out=er_sup[:], out_offset=None,
                        in_=T[:, :],
                        in_offset=IndirectOffsetOnAxis(
                            ap=nid_sb[:, sp:sp + 1], axis=0),
                        element_offset=HD + H,
                    )

                    # HW contract: one index per dest partition-row per
                    # indirect DMA -> one gather per 128-edge chunk.
                    g = gp.tile([CHE, SUP_CH * TCOLS], F32)
                    for i in range(SUP_CH):
                        nc.gpsimd.indirect_dma_start(
                            out=g[:, i * TCOLS:(i + 1) * TCOLS], out_offset=None,
                            in_=T[:, :],
                            in_offset=IndirectOffsetOnAxis(
                                ap=ed_src[:, i:i + 1], axis=0),
                        )
                    gv = g[:].rearrange("p (c w) -> p c w", w=TCOLS)

                    P_t = ppool.tile([CHE, SUP_CH * S], F32)
                    nc.vector.tensor_tensor(
                        out=P_t[:].rearrange("p (c s) -> p c s", s=S),
                        in0=iot[:, None, :].broadcast_to([CHE, SUP_CH, S]),
                        in1=ed_seg[:, :, None].broadcast_to([CHE, SUP_CH, S]),
                        op=AOT.is_equal,
                    )

                    # expand er from slots to edges: PT one-hot + tiny matmul
                    u = exup.tile([CHE, SUP_CH * H], F32, tag="u")
                    for i in range(SUP_CH):
                        pt = ptp.tile([SUP_B * S, CHE], F32)
                        nc.vector.tensor_scalar(
                            out=pt[:], in0=sgt[:, i * CHE:(i + 1) * CHE],
                            scalar1=iot_colf[:, 0:1], scalar2=None,
                            op0=AOT.is_equal)
                        erp_ps = erps.tile([CHE, H], F32)
                        nc.tensor.matmul(
                            out=erp_ps[:], lhsT=pt[:],
                            rhs=er_sup[:],
                            start=True, stop=True)
                        nc.vector.tensor_add(
                            out=u[:, i * H:(i + 1) * H],
                            in0=gv[:, i, HD:HD + H],
                            in1=erp_ps[:],
                        )
                    u2 = exup.tile([CHE, SUP_CH * H], F32, tag="u2")
                    nc.vector.scalar_tensor_tensor(
                        out=u2[:], in0=u[:], scalar=NEG, in1=u[:],
                        op0=AOT.mult, op1=AOT.max)
                    ex = exup.tile([CHE, SUP_CH * H], F32, tag="ex")
                    nc.scalar.activation(
                        out=ex[:], in_=u2[:],
                        func=mybir.ActivationFunctionType.Exp)
                    exv = ex[:].rearrange("p (c h) -> p c h", h=H)

                    mx = mxp.tile([CHE, SUP_CH * ML], F32)
                    mv = mx[:].rearrange("p (c w) -> p c w", w=ML)
                    nc.vector.tensor_copy(out=mv[:, :, HD:HD + H], in_=exv)
                    for h in range(H):
                        nc.vector.tensor_tensor(
                            out=mv[:, :, h * D:(h + 1) * D],
                            in0=gv[:, :, h * D:(h + 1) * D],
                            in1=exv[:, :, h:h + 1].broadcast_to([CHE, SUP_CH, D]),
                            op=AOT.mult,
                        )

                    stg = st2p.tile([S, SUP_B * HD], F32)
                    for j in range(SUP_B):
                        ps = p2ps.tile([S, ML], F32)
                        for cl in range(KC):
                            c = j * KC + cl
                            nc.tensor.matmul(
                                out=ps[:],
                                lhsT=P_t[:, c * S:(c + 1) * S],
                                rhs=mx[:, c * ML:(c + 1) * ML],
                                start=(cl == 0), stop=(cl == KC - 1),
                            )
                        r0 = rp.tile([S, H], F32, tag="r0")
                        nc.vector.tensor_scalar_max(r0[:], ps[:, HD:HD + H], 1e-30)
                        r1 = rp.tile([S, H], F32, tag="r1")
                        nc.vector.reciprocal(r1[:], r0[:])
                        nc.vector.tensor_tensor(
                            out=stg[:, j * HD:(j + 1) * HD].rearrange(
                                "p (h d) -> p h d", h=H),
                            in0=ps[:, 0:HD].rearrange("p (h d) -> p h d", h=H),
                            in1=r1[:, :, None].broadcast_to([S, H, D]),
                            op=AOT.mult,
                        )
                    nc.sync.dma_start(
                        out=out[sp * SUP_B * S:(sp + 1) * SUP_B * S, :].rearrange(
                            "(j p) c -> p j c", j=SUP_B),
                        in_=stg[:].rearrange("p (j c) -> p j c", j=SUP_B),
                    )

    nc.compile()
    return nc


_NC_CACHE = {}
LAST_RESULTS = None


def _get_program(B):
    if B not in _NC_CACHE:
        _NC_CACHE[B] = _build(B)
    return _NC_CACHE[B]


def kernel(feat, fc_w, attn_l, attn_r, src, dst):
    from concourse.bass_utils import run_bass_kernel_spmd

    feat = np.asarray(feat, dtype=np.float32)
    fc_w = np.ascontiguousarray(np.asarray(fc_w, dtype=np.float32))
    attn_l = np.asarray(attn_l, dtype=np.float32)
    attn_r = np.asarray(attn_r, dtype=np.float32)
    src = np.asarray(src).astype(np.int64)
    dst = np.asarray(dst).astype(np.int64)
    n_edges = src.shape[0]

    edatas, node_maps, B = _pack(src, dst, NCORES, N, n_edges)

    featT = np.zeros((F, NPAD), np.float32)
    featT[:, :N] = feat.T
    featT = np.ascontiguousarray(featT)
    attn = np.concatenate(
        [attn_l.reshape(-1), attn_r.reshape(-1)]).reshape(1, 2 * HD)
    attn = np.ascontiguousarray(attn.astype(np.float32))

    nc = _get_program(B)
    in_maps = [
        {"featT": featT, "fcw": fc_w, "attn": attn,
         "edata": edatas[k][0], "nid": edatas[k][1], "segT": edatas[k][2]}
        for k in range(NCORES)
    ]
    res = run_bass_kernel_spmd(nc, in_maps, core_ids=list(range(NCORES)))
    global LAST_RESULTS
    LAST_RESULTS = res

    outf = np.zeros((N, HD), np.float32)
    for k in range(NCORES):
        o = np.asarray(res.results[k]["out"])
        nm = node_maps[k]
        m = nm >= 0
        outf[nm[m]] = o[m]
    return outf



# revision 2
# speedup vs baseline: 1.0767x; 1.0767x over previous
"""GAT message-passing kernel for Trainium2 (8 NeuronCores, Bass/Tile). v2

Same structure as v1 (edge/graph parallelism, dst-sorted blocks, one-hot
PSUM aggregation), with the memory diet:
  - T table, gathers, and all matmuls in bf16 (PSUM accumulation stays fp32).
  - segT (the pt one-hot helper) in int8 instead of fp32 (4x less DMA).
  - phase-1 inputs (featT / fc_w / attn) pre-cast to bf16 on the host.
"""

import math
import numpy as np

N = 100000
F = 128
H = 4
D = 32
HD = H * D        # 128
TCOLS = F + 2 * H  # 136
ML = HD + H       # 132
NEG = 0.2
NCORES = 8

S = 32
KC = 4
CHE = 128
BSLOTS = KC * CHE
SUP_B = 4
SUP_CH = SUP_B * KC
PAD_SEG = 100000

NPAD = 100352
WCH = 2048
G1 = 8


def _pack(src, dst, n_cores, n_nodes, n_edges):
    """Host-side index preprocessing. Returns (edata list, node_map list, B)."""
    order = np.argsort(dst, kind="stable")
    s_src = np.asarray(src, np.int64)[order]
    s_dst = np.asarray(dst, np.int64)[order]
    deg = np.bincount(dst, minlength=n_nodes).astype(np.int64)
    assert deg.max() <= BSLOTS, "node degree exceeds block capacity"
    cum = np.cumsum(deg)
    estart = cum - deg
    bnd = [0]
    for k in range(1, n_cores):
        bnd.append(int(np.searchsorted(cum, n_edges * k / n_cores)))
    bnd.append(n_nodes)

    node_block = np.zeros(n_nodes, np.int64)
    node_slot = np.zeros(n_nodes, np.int64)
    nblocks = []
    for k in range(n_cores):
        nb = 0
        cnt = 0
        slots = 0
        for n in range(bnd[k], bnd[k + 1]):
            d = deg[n]
            if cnt >= S or slots + d > BSLOTS:
                nb += 1
                cnt = 0
                slots = 0
            node_block[n] = nb
            node_slot[n] = cnt
            cnt += 1
            slots += d
        nblocks.append(nb + 1 if bnd[k + 1] > bnd[k] else 0)
    B = max(nblocks)
    B = int(math.ceil(B / SUP_B) * SUP_B)

    edatas = []
    node_maps = []
    for k in range(n_cores):
        lo, hi = bnd[k], bnd[k + 1]
        e_lo = int(estart[lo]) if lo < n_nodes else n_edges
        e_hi = int(estart[hi]) if hi < n_nodes else n_edges
        ksrc = s_src[e_lo:e_hi]
        kdst = s_dst[e_lo:e_hi]
        kblk = node_block[kdst]
        kslot = node_slot[kdst]
        nodes = np.arange(lo, hi)
        blk_of_node = node_block[lo:hi]
        nb_k = nblocks[k]
        first_edge = np.zeros(max(nb_k, 1), np.int64)
        ub, ui = np.unique(blk_of_node, return_index=True)
        first_edge[ub] = estart[nodes[ui]]
        pos = np.arange(e_lo, e_hi) - first_edge[kblk]
        assert pos.max(initial=0) < BSLOTS
        c = kblk * KC + pos // CHE
        p = pos % CHE
        sp_of_c = c // SUP_CH
        i_of_c = c % SUP_CH
        base = sp_of_c * 3 * SUP_CH
        ed = np.zeros((CHE, B * KC * 3), np.int32)
        seg_cols = (np.arange(B * KC * 3)
                    .reshape(-1, 3 * SUP_CH)[:, SUP_CH:2 * SUP_CH].reshape(-1))
        ed[:, seg_cols] = PAD_SEG
        ed[p, base + i_of_c] = ksrc
        ed[p, base + SUP_CH + i_of_c] = kslot
        ed[p, base + 2 * SUP_CH + i_of_c] = kdst
        nm = np.full(B * S, -1, np.int64)
        nm[blk_of_node * S + node_slot[lo:hi]] = nodes
        nsup_k = B // SUP_B
        nid = np.zeros((SUP_B * S, nsup_k), np.int32)
        nid[(blk_of_node % SUP_B) * S + node_slot[lo:hi],
            blk_of_node // SUP_B] = nodes
        # int8 slot-id rows (pad = -1 matches no slot in 0..127)
        segT = np.full((1, B * KC * CHE), -1, np.int8)
        segT[0, c * CHE + p] = (kslot + S * (kblk % SUP_B)).astype(np.int8)
        segT = np.broadcast_to(segT, (SUP_B * S, B * KC * CHE)).copy()
        edatas.append((ed, nid, segT))
        node_maps.append(nm)
    return edatas, node_maps, B


def _build(B, npad=NPAD, wch=WCH, g1=G1):
    import concourse.bacc as bacc
    import concourse.tile as tile
    import concourse.mybir as mybir
    from concourse.bass import IndirectOffsetOnAxis

    F32 = mybir.dt.float32
    BF = mybir.dt.bfloat16
    I32 = mybir.dt.int32
    I8 = mybir.dt.int8
    AOT = mybir.AluOpType

    nsup = B // SUP_B

    nc = bacc.Bacc("TRN2", target_bir_lowering=False, debug=False)
    featT = nc.dram_tensor("featT", [F, npad], BF, kind="ExternalInput")
    fcw = nc.dram_tensor("fcw", [F, HD], BF, kind="ExternalInput")
    attn = nc.dram_tensor("attn", [1, 2 * HD], BF, kind="ExternalInput")
    edata = nc.dram_tensor("edata", [CHE, B * KC * 3], I32, kind="ExternalInput")
    nid_d = nc.dram_tensor("nid", [SUP_B * S, B // SUP_B], I32, kind="ExternalInput")
    segT_d = nc.dram_tensor("segT", [SUP_B * S, B * KC * CHE], I8, kind="ExternalInput")
    T = nc.dram_tensor("T", [npad, TCOLS], BF, kind="Internal")
    out = nc.dram_tensor("out", [B * S, HD], F32, kind="ExternalOutput")

    with tile.TileContext(nc) as tc, \
         nc.allow_low_precision(reason="bf16 gather/message pipeline, fp32 PSUM"):
        with tc.tile_pool(name="const", bufs=1) as const:
            # ---- weight prep: W_aug = [fc_w | W_l | W_r] (bf16) ----
            w_aug = const.tile([F, TCOLS], BF)
            nc.sync.dma_start(out=w_aug[:, 0:HD], in_=fcw[:, :])
            attn_sb = const.tile([1, 2 * HD], BF)
            nc.sync.dma_start(out=attn_sb[:], in_=attn[:, :])
            ab = const.tile([F, 2 * HD], BF)
            nc.gpsimd.partition_broadcast(ab[:], attn_sb[:])
            tmp = const.tile([F, 2 * HD], F32)
            nc.vector.tensor_tensor(
                out=tmp[:].rearrange("p (t w) -> p t w", t=2),
                in0=w_aug[:, None, 0:HD].broadcast_to([F, 2, HD]),
                in1=ab[:].rearrange("p (t w) -> p t w", t=2),
                op=AOT.mult,
            )
            nc.vector.tensor_reduce(
                w_aug[:, HD:HD + 2 * H].rearrange("p (t h) -> p t h", t=2),
                tmp[:].rearrange("p (t h d) -> p t h d", t=2, h=H),
                mybir.AxisListType.X,
                AOT.add,
            )

            # ---- phase 1: T = [feat @ W_aug] (bf16 in/out, fp32 PSUM) ----
            with tc.tile_pool(name="fp", bufs=3) as fpool, \
                 tc.tile_pool(name="p1ps", bufs=8, space="PSUM") as p1ps, \
                 tc.tile_pool(name="st1", bufs=4) as st1p:
                tpw = wch // 128
                for w in range(npad // wch):
                    fsb = fpool.tile([F, wch], BF)
                    nc.sync.dma_start(out=fsb[:], in_=featT[:, w * wch:(w + 1) * wch])
                    for grp in range(tpw // g1):
                        stg = st1p.tile([F, g1 * TCOLS], BF)
                        for j in range(g1):
                            ps = p1ps.tile([128, TCOLS], F32)
                            col0 = (grp * g1 + j) * 128
                            nc.tensor.matmul(
                                out=ps[:],
                                lhsT=fsb[:, col0:col0 + 128],
                                rhs=w_aug[:],
                                start=True, stop=True,
                            )
                            nc.vector.tensor_copy(
                                out=stg[:, j * TCOLS:(j + 1) * TCOLS], in_=ps[:]
                            )
                        t0 = w * tpw + grp * g1
                        nc.sync.dma_start(
                            out=T[t0 * 128:(t0 + g1) * 128, :].rearrange(
                                "(j p) c -> p j c", j=g1),
                            in_=stg[:].rearrange("p (j c) -> p j c", j=g1),
                        )

            # ---- phase 2: edge processing ----
            iot = const.tile([CHE, S], I32)
            nc.gpsimd.iota(iot[:], pattern=[[1, S]], base=0, channel_multiplier=0)
            iot_col = const.tile([SUP_B * S, 1], I32)
            nc.gpsimd.iota(iot_col[:], pattern=[[0, 1]], base=0,
                           channel_multiplier=1)
            iot_colf = const.tile([SUP_B * S, 1], F32)
            nc.vector.tensor_copy(out=iot_colf[:], in_=iot_col[:])
            nid_sb = const.tile([SUP_B * S, nsup], I32)
            nc.sync.dma_start(out=nid_sb[:], in_=nid_d[:, :])

            with tc.tile_pool(name="ed", bufs=5) as edp, \
                 tc.tile_pool(name="gg", bufs=5) as gp, \
                 tc.tile_pool(name="sgt", bufs=4) as sgtp, \
                 tc.tile_pool(name="ers", bufs=6) as ersp, \
                 tc.tile_pool(name="pp", bufs=3) as ppool, \
                 tc.tile_pool(name="pt", bufs=4) as ptp, \
                 tc.tile_pool(name="mx", bufs=3) as mxp, \
                 tc.tile_pool(name="exu", bufs=3) as exup, \
                 tc.tile_pool(name="rr", bufs=8) as rp, \
                 tc.tile_pool(name="st2", bufs=3) as st2p, \
                 tc.tile_pool(name="p2ps", bufs=6, space="PSUM") as p2ps, \
                 tc.tile_pool(name="erps", bufs=2, space="PSUM") as erps:
                for sp in range(nsup):
                    ed = edp.tile([CHE, SUP_CH * 3], I32)
                    nc.sync.dma_start(
                        out=ed[:],
                        in_=edata[:, sp * SUP_CH * 3:(sp + 1) * SUP_CH * 3])
                    ed_src = ed[:, 0:SUP_CH]
                    ed_seg = ed[:, SUP_CH:2 * SUP_CH]

                    sgt = sgtp.tile([SUP_B * S, SUP_CH * CHE], I8)
                    nc.sync.dma_start(
                        out=sgt[:],
                        in_=segT_d[:, sp * SUP_CH * CHE:(sp + 1) * SUP_CH * CHE])

                    er_sup = ersp.tile([SUP_B * S, H], BF)
                    nc.gpsimd.indirect_dma_start(
                        out=er_sup[:], out_offset=None,
                        in_=T[:, :],
                        in_offset=IndirectOffsetOnAxis(
                            ap=nid_sb[:, sp:sp + 1], axis=0),
                        element_offset=HD + H,
                    )

                    g = gp.tile([CHE, SUP_CH * TCOLS], BF)
                    for i in range(SUP_CH):
                        nc.gpsimd.indirect_dma_start(
                            out=g[:, i * TCOLS:(i + 1) * TCOLS], out_offset=None,
                            in_=T[:, :],
                            in_offset=IndirectOffsetOnAxis(
                                ap=ed_src[:, i:i + 1], axis=0),
                        )
                    gv = g[:].rearrange("p (c w) -> p c w", w=TCOLS)

                    P_t = ppool.tile([CHE, SUP_CH * S], BF)
                    nc.vector.tensor_tensor(
                        out=P_t[:].rearrange("p (c s) -> p c s", s=S),
                        in0=iot[:, None, :].broadcast_to([CHE, SUP_CH, S]),
                        in1=ed_seg[:, :, None].broadcast_to([CHE, SUP_CH, S]),
                        op=AOT.is_equal,
                    )

                    # er expansion: int8 segT -> pt one-hot (bf16) + tiny matmul
                    u = exup.tile([CHE, SUP_CH * H], F32, tag="u")
                    for i in range(SUP_CH):
                        pt = ptp.tile([SUP_B * S, CHE], BF)
                        nc.vector.tensor_scalar(
                            out=pt[:], in0=sgt[:, i * CHE:(i + 1) * CHE],
                            scalar1=iot_colf[:, 0:1], scalar2=None,
                            op0=AOT.is_equal)
                        erp_ps = erps.tile([CHE, H], F32)
                        nc.tensor.matmul(
                            out=erp_ps[:], lhsT=pt[:],
                            rhs=er_sup[:],
                            start=True, stop=True)
                        nc.vector.tensor_add(
                            out=u[:, i * H:(i + 1) * H],
                            in0=gv[:, i, HD:HD + H],
                            in1=erp_ps[:],
                        )
                    u2 = exup.tile([CHE, SUP_CH * H], F32, tag="u2")
                    nc.vector.scalar_tensor_tensor(
                        out=u2[:], in0=u[:], scalar=NEG, in1=u[:],
                        op0=AOT.mult, op1=AOT.max)
                    ex = exup.tile([CHE, SUP_CH * H], BF, tag="ex")
                    nc.scalar.activation(
                        out=ex[:], in_=u2[:],
                        func=mybir.ActivationFunctionType.Exp)
                    exv = ex[:].rearrange("p (c h) -> p c h", h=H)

                    mx = mxp.tile([CHE, SUP_CH * ML], BF)
                    mv = mx[:].rearrange("p (c w) -> p c w", w=ML)
                    nc.vector.tensor_copy(out=mv[:, :, HD:HD + H], in_=exv)
                    for h in range(H):
                        nc.vector.tensor_tensor(
                            out=mv[:, :, h * D:(h + 1) * D],
                            in0=gv[:, :, h * D:(h + 1) * D],
                            in1=exv[:, :, h:h + 1].broadcast_to([CHE, SUP_CH, D]),
                            op=AOT.mult,
                        )

                    stg = st2p.tile([S, SUP_B * HD], F32)
                    for j in range(SUP_B):
                        ps = p2ps.tile([S, ML], F32)
                        for cl in range(KC):
                            c = j * KC + cl
                            nc.tensor.matmul(
                                out=ps[:],
                                lhsT=P_t[:, c * S:(c + 1) * S],
                                rhs=mx[:, c * ML:(c + 1) * ML],
                                start=(cl == 0), stop=(cl == KC - 1),
                            )
                        r0 = rp.tile([S, H], F32, tag="r0")
                        nc.vector.tensor_scalar_max(r0[:], ps[:, HD:HD + H], 1e-30)
                        r1 = rp.tile([S, H], F32, tag="r1")
                        nc.vector.reciprocal(r1[:], r0[:])
                        nc.vector.tensor_tensor(
                            out=stg[:, j * HD:(j + 1) * HD].rearrange(
                                "p (h d) -> p h d", h=H),
                            in0=ps[:, 0:HD].rearrange("p (h d) -> p h d", h=H),
                            in1=r1[:, :, None].broadcast_to([S, H, D]),
                            op=AOT.mult,
                        )
                    nc.sync.dma_start(
                        out=out[sp * SUP_B * S:(sp + 1) * SUP_B * S, :].rearrange(
                            "(j p) c -> p j c", j=SUP_B),
                        in_=stg[:].rearrange("p (j c) -> p j c", j=SUP_B),
                    )

    nc.compile()
    return nc


_NC_CACHE = {}
LAST_RESULTS = None


def _get_program(B):
    if B not in _NC_CACHE:
        _NC_CACHE[B] = _build(B)
    return _NC_CACHE[B]


def kernel(feat, fc_w, attn_l, attn_r, src, dst):
    import ml_dtypes
    from concourse.bass_utils import run_bass_kernel_spmd

    BF = ml_dtypes.bfloat16
    feat = np.asarray(feat, dtype=np.float32)
    fc_w = np.ascontiguousarray(np.asarray(fc_w, dtype=np.float32))
    attn_l = np.asarray(attn_l, dtype=np.float32)
    attn_r = np.asarray(attn_r, dtype=np.float32)
    src = np.asarray(src).astype(np.int64)
    dst = np.asarray(dst).astype(np.int64)
    n_edges = src.shape[0]

    edatas, node_maps, B = _pack(src, dst, NCORES, N, n_edges)

    featT = np.zeros((F, NPAD), np.float32)
    featT[:, :N] = feat.T
    featT = np.ascontiguousarray(featT).astype(BF)
    fcw_bf = fc_w.astype(BF)
    attn = np.concatenate(
        [attn_l.reshape(-1), attn_r.reshape(-1)]).reshape(1, 2 * HD)
    attn = np.ascontiguousarray(attn.astype(BF))

    nc = _get_program(B)
    in_maps = [
        {"featT": featT, "fcw": fcw_bf, "attn": attn,
         "edata": edatas[k][0], "nid": edatas[k][1], "segT": edatas[k][2]}
        for k in range(NCORES)
    ]
    res = run_bass_kernel_spmd(nc, in_maps, core_ids=list(range(NCORES)))
    global LAST_RESULTS
    LAST_RESULTS = res

    outf = np.zeros((N, HD), np.float32)
    for k in range(NCORES):
        o = np.asarray(res.results[k]["out"])
        nm = node_maps[k]
        m = nm >= 0
        outf[nm[m]] = o[m]
    return outf
